# revision 8
# baseline (speedup 1.0000x reference)
"""ConvBert LightConv kernel for Trainium2 (Bass/Tile), batch-parallel on 8 cores.

out[b,s,h,c] = sum_j softmax_j(filters[b,s,h,:])[j] * x_pad[b, s+j-4, h*64+c]

Per-core algorithm (core owns one batch, [S=4096, D=768]):

  PREPASS (8 iterations x 512 tokens): softmax-normalize the filters
  (ACT exp, DVE reduce+reciprocal+normalize -> bf16), then ONE DMA per
  iteration writes the staggered layout
        Lp[8 + s + j, j, h] = fn[s, h, j]
  using a fused-stride access pattern (row index advances with both the
  token and the tap, so the whole 9-tap stagger is a single linear AP:
  strides k:108, block:13824, j:120, h:1).  Zero-filled pad rows cover
  the (row, tap) combinations outside the valid band.

  MAIN (35 tiles x TW=120 output tokens, x window = 128 rows [t0-4, t0+124)):
    - x DMA fp32; ACT-convert to bf16 with fused (h,c)->(c,h) reorder so
      every product operand is packed 2-byte innermost -> DVE 2x mode.
    - fn_s DMA: ONE contiguous read Lp[8+t0 : 8+t0+128] gives, for each
      tap j, exactly the per-token filter scalars the products need
      (fn_s[k, j, :] = fn[t0+k-j, :, j] -- the stagger baked the shift in).
    - products P[k, j, ch] = x_c[k, ch] * fn_s[k, j, h]: 7 taps on DVE
      (2x mode, 424ns), 2 on GPSIMD.  All ops partition-base 0 (HW rule:
      engine APs must start at partition 0/32/64).
    - tap-sum on PE: out[m] = sum_j P[m+j, j] as 9 accumulating PSUM
      matmuls with static 0/1 shift stationaries S_j[k,m] = (k==m+j),
      bf16 -> 1 cycle/row (4x faster than fp32 matmuls).
    - evacuate: ACT copy PSUM->SBUF fused with the (c,h)->(h,c)
      un-reorder, fp32; one DMA out.

  Out-of-band garbage entering the matmuls is finite (zero-filled Lp pads,
  memset x edges) and lands on all-zero stationary columns -> contributes
  exact 0.

~130 DMAs total vs 723 in the staged fp32 design.
"""

import os
import sys

import numpy as np

for _p in ("/opt/trn_rl_repo",):
    if _p not in sys.path:
        sys.path.insert(0, _p)

B, S, D = 8, 4096, 768
H, HD, KS = 12, 64, 9
FH = H * KS  # 108 filter scalars per token
PAD = KS // 2  # 4
TW = 120  # output tokens per main tile (x window = TW + 8 = 128 rows)
NT = (S + TW - 1) // TW  # 35 tiles; last covers 16 tokens
PB = 4  # 128-token blocks per prepass iteration
NP = S // (128 * PB)  # 8 prepass iterations
LROWS = 8 + S + 136  # staggered rows: pad(8) + S + tail pad

_CACHE = {}


def _build_program():
    import ml_dtypes

    import concourse.bass as bass
    import concourse.tile as tile
    from concourse import mybir

    f32 = mybir.dt.float32
    bf16 = mybir.dt.bfloat16
    Exp = mybir.ActivationFunctionType.Exp
    Copy = mybir.ActivationFunctionType.Copy

    nc = bass.Bass()
    x_d = nc.dram_tensor("x", [S, D], f32, kind="ExternalInput")
    f_d = nc.dram_tensor("f", [S, FH], f32, kind="ExternalInput")
    o_d = nc.dram_tensor("o", [S, D], f32, kind="ExternalOutput")

    # Static shift stationaries, [k=128, j, m=120] with sh[m+j, j, m] = 1.
    sh_np = np.zeros((128, KS, TW), dtype=ml_dtypes.bfloat16)
    for j in range(KS):
        for m in range(TW):
            sh_np[m + j, j, m] = 1.0
    sh_d = nc.inline_tensor(sh_np, name="shift_mats")

    with tile.TileContext(nc) as tc:
        with (
            tc.tile_pool(name="singles", bufs=1) as singles,
            tc.tile_pool(name="dram", bufs=1, space="DRAM") as dpool,
            tc.tile_pool(name="pre", bufs=2) as pre,
            tc.tile_pool(name="xin", bufs=3) as xin,
            tc.tile_pool(name="xcv", bufs=3) as xcv,
            tc.tile_pool(name="fst", bufs=3) as fst,
            tc.tile_pool(name="prod", bufs=3) as prod,
            tc.tile_pool(name="outs", bufs=3) as outs,
            tc.tile_pool(name="ps", bufs=3, space="PSUM") as ps,
        ):
            lp = dpool.tile([LROWS, FH], bf16, name="lp_stagger")

            s_sb = singles.tile([128, KS, TW], bf16)
            nc.sync.dma_start(out=s_sb, in_=sh_d[:, :, :])

            # Zero-fill Lp rows that the prepass never fully covers BEFORE
            # the prepass (written combos are then overwritten): head rows
            # [0, 16) and tail rows [8+S, LROWS).
            zro = singles.tile([128, FH], bf16)
            nc.gpsimd.memset(zro, 0.0)
            nc.sync.dma_start(out=lp[0:16, :], in_=zro[0:16, :])
            nc.sync.dma_start(out=lp[8 + S : 8 + S + 128, :], in_=zro)
            nc.sync.dma_start(
                out=lp[8 + S + 128 : LROWS, :], in_=zro[0 : LROWS - 8 - S - 128, :]
            )

            # ---- softmax prepass: 8 iterations x (4 blocks x 128 tokens),
            # interleaved into the main loop so main-tile DMAs are not
            # queued behind the whole prepass on the FIFO DMA queue ----
            def emit_prepass(it):
                r0 = 512 * it
                f_t = pre.tile([128, PB, H, KS], f32, tag="f_t")
                src = f_d[r0 : r0 + 128, :].copy()
                src.ap = type(src.ap)([[FH, 128], [128 * FH, PB], [1, FH]])
                nc.sync.dma_start(
                    out=f_t.rearrange("p b h j -> p b (h j)"), in_=src
                )
                e_t = pre.tile([128, PB, H, KS], f32, tag="e_t")
                nc.scalar.activation(e_t, f_t, Exp)
                z_t = pre.tile([128, PB, H], f32, tag="z_t")
                nc.vector.tensor_reduce(
                    out=z_t,
                    in_=e_t,
                    axis=mybir.AxisListType.X,
                    op=mybir.AluOpType.add,
                )
                r_t = pre.tile([128, PB, H], f32, tag="r_t")
                nc.vector.reciprocal(r_t, z_t)
                fn_t = pre.tile([128, PB, KS, H], bf16, tag="fn_t")
                nc.vector.tensor_mul(
                    fn_t,
                    e_t.rearrange("p b h j -> p b j h"),
                    r_t.unsqueeze(2).broadcast_to([128, PB, KS, H]),
                )
                # Staggered writes (one per 128-token block):
                #   Lp[8+r0+128b+k+j, j*12+h] = fn_t[k, b, j, h]
                # The row index advances with both k and j, so the whole
                # 9-tap stagger is one linear 3-dim AP per block.
                for b in range(PB):
                    dst = lp[0:128, :].copy()
                    dst.ap = type(dst.ap)([[FH, 128], [FH + H, KS], [1, H]])
                    dst.offset = (8 + r0 + 128 * b) * FH
                    nc.sync.dma_start(out=dst, in_=fn_t[:, b])

            # ---- main pass ----
            next_it = 0
            for t in range(NT):
                # keep the stagger buffer one prepass iteration ahead
                need_it = min(NP - 1, (TW * t + 127) // 512 + 1)
                while next_it <= need_it:
                    emit_prepass(next_it)
                    next_it += 1
                t0 = TW * t
                tw = min(TW, S - t0)  # valid out tokens (16 on last tile)
                u0 = t0 - PAD  # first x row of this tile's window

                # staggered filter read: fn_s[k, j, h] = fn[t0+k-j, h, j]
                fn_s = fst.tile([128, KS, H], bf16, tag="fn_s")
                nc.sync.dma_start(
                    out=fn_s, in_=lp[8 + t0 : 8 + t0 + 128, :]
                )

                # x window (fp32)
                x_t = xin.tile([128, D], f32, tag="x_t")
                if t == 0:
                    nc.gpsimd.memset(x_t[0:PAD, :], 0.0)
                    nc.sync.dma_start(out=x_t[PAD:128, :], in_=x_d[0 : 128 - PAD, :])
                elif u0 + 128 > S:
                    nv = S - u0
                    nc.gpsimd.memset(x_t, 0.0)
                    nc.sync.dma_start(out=x_t[0:nv, :], in_=x_d[u0:S, :])
                else:
                    nc.sync.dma_start(out=x_t, in_=x_d[u0 : u0 + 128, :])

                # x -> bf16, channel-major (c,h)
                x_c = xcv.tile([128, HD, H], bf16, tag="x_c")
                nc.scalar.activation(
                    x_c, x_t.rearrange("p (h c) -> p c h", c=HD), Copy
                )

                # products: bf16, full 128 rows, all partition-base 0
                p_t = prod.tile([128, KS, D], bf16, tag="p_t")
                for j in range(KS):
                    eng = nc.gpsimd if j >= 7 else nc.vector
                    eng.tensor_mul(
                        p_t[:, j].rearrange("p (c h) -> p c h", h=H),
                        x_c,
                        fn_s[:, j].unsqueeze(1).broadcast_to([128, HD, H]),
                    )

                # tap-sum: accumulating shift matmuls into PSUM
                o_ps = ps.tile([128, D], f32, tag="o_ps")
                for j in range(KS):
                    for n0, n1 in ((0, 512), (512, D)):
                        nc.tensor.matmul(
                            o_ps[0:TW, n0:n1],
                            s_sb[:, j, :],
                            p_t[:, j, n0:n1],
                            start=(j == 0),
                            stop=(j == KS - 1),
                        )

                # evacuate + (c,h)->(h,c) un-reorder, fp32
                o_t = outs.tile([128, D], f32, tag="o_t")
                nc.scalar.activation(
                    o_t[0:tw].rearrange("p (h c) -> p h c", c=HD),
                    o_ps[0:tw].rearrange("p (c h) -> p h c", h=H),
                    Copy,
                )
                nc.sync.dma_start(out=o_d[t0 : t0 + tw, :], in_=o_t[0:tw, :])

    _split_hwdge_multi_waits(nc)
    return nc


def _split_hwdge_multi_waits(nc):
    """walrus's HWDGE DMA trigger (PSEUDO_DMA_DIRECT2D) rejects >1 sync wait
    on a DMACopy. Move all but one wait onto a NoOp inserted right before the
    DMA on the same (sequencer) engine — identical semantics, since the
    sequencer executes both in order before triggering the descriptor."""
    from concourse import mybir

    nsplit = 0
    for fn in nc.m.functions:
        for blk in fn.blocks:
            out = []
            for ins in blk.instructions:
                si = ins.sync_info
                if si is not None and len(si.on_wait) > 1:
                    for wi, w in enumerate(si.on_wait[:-1]):
                        nop = mybir.InstNoOp(
                            name=f"{ins.name}_waitsplit{wi}",
                            engine=ins.engine,
                            sync_info=mybir.SyncInfo(on_wait=[w], on_update=[]),
                        )
                        out.append(nop)
                    ins.sync_info = mybir.SyncInfo(
                        on_wait=list(si.on_wait[-1:]),
                        on_update=list(si.on_update),
                    )
                    nsplit += 1
                out.append(ins)
            blk.instructions = out
    if nsplit and os.environ.get("LC_DEBUG"):
        print(f"_split_hwdge_multi_waits: split {nsplit} DMAs")


def kernel(inputs: np.ndarray, filters: np.ndarray) -> np.ndarray:
    from concourse.bass_utils import run_bass_kernel_spmd

    if "nc" not in _CACHE:
        _CACHE["nc"] = _build_program()
    nc = _CACHE["nc"]

    inputs = np.ascontiguousarray(np.asarray(inputs, dtype=np.float32))
    filters = np.ascontiguousarray(np.asarray(filters, dtype=np.float32))

    in_maps = [{"x": inputs[c], "f": filters[c]} for c in range(B)]

    res = run_bass_kernel_spmd(nc, in_maps, core_ids=list(range(B)), trace=False)

    out = np.stack([res.results[c]["o"] for c in range(B)], axis=0)
    return out.reshape(B, S, H, HD)


def bench(inputs: np.ndarray, filters: np.ndarray, reps: int = 20) -> float:
    """Device-resident repeated execution; returns mean seconds per call
    (includes PJRT dispatch, excludes host<->device transfer)."""
    import time

    import jax
    from jax.experimental.shard_map import shard_map
    from jax.sharding import Mesh, PartitionSpec

    import concourse.mybir as mybir
    from concourse import bass2jax

    if "nc" not in _CACHE:
        _CACHE["nc"] = _build_program()
    nc = _CACHE["nc"]
    bass2jax.install_neuronx_cc_hook()

    part_name = nc.partition_id_tensor.name if nc.partition_id_tensor else None
    in_names, out_names, out_avals, zero_outs = [], [], [], []
    for alloc in nc.m.functions[0].allocations:
        if not isinstance(alloc, mybir.MemoryLocationSet):
            continue
        name = alloc.memorylocations[0].name
        if alloc.kind == "ExternalInput":
            if name != part_name:
                in_names.append(name)
        elif alloc.kind == "ExternalOutput":
            out_names.append(name)
            shape = tuple(alloc.tensor_shape)
            dtype = mybir.dt.np(alloc.dtype)
            out_avals.append(jax.core.ShapedArray(shape, dtype))
            zero_outs.append(np.zeros(shape, dtype))
    n_params = len(in_names)
    all_names = in_names + out_names
    if part_name is not None:
        all_names = all_names + [part_name]

    def _body(*args):
        operands = list(args)
        if part_name is not None:
            operands.append(bass2jax.partition_id_tensor())
        outs = bass2jax._bass_exec_p.bind(
            *operands,
            out_avals=tuple(out_avals),
            in_names=tuple(all_names),
            out_names=tuple(out_names),
            lowering_input_output_aliases=(),
            sim_require_finite=True,
            sim_require_nnan=True,
            nc=nc,
        )
        return tuple(outs)

    devices = jax.devices()[:B]
    mesh = Mesh(np.asarray(devices), ("core",))
    nin = n_params + len(out_names)
    fn = jax.jit(
        shard_map(
            _body,
            mesh=mesh,
            in_specs=(PartitionSpec("core"),) * nin,
            out_specs=(PartitionSpec("core"),) * len(out_names),
            check_rep=False,
        ),
        keep_unused=True,
    )
    per_core = {"x": inputs.astype(np.float32), "f": filters.astype(np.float32)}
    concat_in = [
        np.concatenate([per_core[n][c] for c in range(B)], axis=0) for n in in_names
    ]
    concat_zero = [
        np.zeros((B * z.shape[0], *z.shape[1:]), z.dtype) for z in zero_outs
    ]
    sharding = jax.sharding.NamedSharding(mesh, PartitionSpec("core"))
    dev_args = [jax.device_put(a, sharding) for a in concat_in + concat_zero]

    out = fn(*dev_args)  # compile + warm
    jax.block_until_ready(out)
    t0 = time.perf_counter()
    for _ in range(reps):
        out = fn(*dev_args)
    jax.block_until_ready(out)
    t1 = time.perf_counter()
    return (t1 - t0) / reps


if __name__ == "__main__":
    rng = np.random.default_rng(0)
    x = rng.standard_normal((B, S, D), dtype=np.float32)
    f = rng.standard_normal((B, S, H * KS), dtype=np.float32)
    o = kernel(x, f)
    print(o.shape, o.dtype)
